# revision 7
# baseline (speedup 1.0000x reference)
"""Chamfer distance (squared-L2, mean-min both directions) on 8 Trainium2 cores.

Strategy
--------
B=16 batches of N=M=4096 3-D points -> data-parallel: 2 batches per core.

For each batch and each direction (query=xyz1 vs query=xyz2) the device
computes full distance tiles D[n, m] = |q_n|^2 + |k_m|^2 - 2 q_n.k_m with a
single K=24 stacked matmul: the fp32 coordinates are split host-side into
three bf16 components (hi/mid/lo) and all cross products down to 2^-24
relative magnitude are kept, so the bf16 TensorE (1 cycle/row, 4x faster
than native fp32 matmul) reproduces fp32-accuracy distances in the fp32
PSUM accumulator.  |q|^2 / |k|^2 enter the same matmul as three extra
bf16 rows against a ones-row on the opposite side.

Both reductions become row-min reductions (direction 2 computes D^T), which
lets one DVE tensor_tensor_reduce(min, min) per 128-row chunk consume the
[128, 4096] PSUM stripe at 2 elements/lane/cycle: ScalarE first copies the
left half to SBUF, then the DVE instruction min-pairs the SBUF half with
the PSUM half elementwise and min-reduces the pairs to a [128, 1] row-min
column in the same pass.

The device returns per-core [128, 128] row-min tiles; the host clamps at 0
(identical to the reference's maximum(d, 0) because min and clamp commute)
and averages.
"""

import sys
from contextlib import ExitStack

import numpy as np

sys.path.insert(0, "/opt/trn_rl_repo")

import ml_dtypes

import concourse.bass as bass
import concourse.tile as tile
from concourse import bacc, mybir
from concourse.bass_utils import run_bass_kernel_spmd

B, N, M = 16, 4096, 4096
NCORES = 8
BPC = B // NCORES          # batches per core
NDIR = 2 * BPC             # matmul directions per core (2 per batch)
K = 32                     # stacked contraction rows
NCHUNK = N // 128          # 32 output-row chunks per direction
RM_COLS = NDIR * NCHUNK    # 128 row-min columns per core
HALF = 2048                # half-stripe width (4 PSUM banks)
BF16 = ml_dtypes.bfloat16
BIG = float(np.finfo(np.float32).max)


# ----------------------------------------------------------------- host prep

def _splitn(x, n):
    """x (fp32/fp64) -> n bf16 arrays p_i with sum(p_i) = x + O(2^-(8n) x)."""
    parts = []
    r = x
    for _ in range(n):
        p = r.astype(BF16)
        parts.append(p)
        r = r - p.astype(x.dtype)
    return parts


def _stacks(z):
    """z: [N, 3] fp32 points -> (lhsT_stack [K, N] bf16, rhs_stack [K, N] bf16).

    Row pairing (lhsT row k multiplies rhs row k, summed over k): the 3-way
    bf16 split of each coordinate (h/m/l) keeps all cross products except
    l.l (2^-32 relative); |z|^2 enters as a 4-way bf16 split against a
    ones-row on the opposite side.
      k 0-8  : (-2 h1).(h2|m2|l2)    k 9-17 : (-2 m1).(h2|m2|l2)
      k 18-23: (-2 l1).(h2|m2)       k 24-27: sq1 parts . 1
      k 28-31: 1 . sq2 parts
    """
    zt = np.ascontiguousarray(z.T.astype(np.float32))          # [3, N]
    h, m, l = _splitn(zt, 3)
    sq = (z.astype(np.float64) ** 2).sum(axis=-1)              # [N]
    sqp = _splitn(sq, 4)
    npts = z.shape[0]

    lhs = np.empty((K, npts), dtype=BF16)
    h2 = (-2.0 * h.astype(np.float32)).astype(BF16)            # exact (power of 2)
    m2 = (-2.0 * m.astype(np.float32)).astype(BF16)
    l2 = (-2.0 * l.astype(np.float32)).astype(BF16)
    for i, a in enumerate((h2, h2, h2, m2, m2, m2, l2, l2)):
        lhs[3 * i: 3 * i + 3] = a
    for i in range(4):
        lhs[24 + i] = sqp[i]
    lhs[28:32] = np.ones((4, npts), dtype=BF16)

    rhs = np.empty((K, npts), dtype=BF16)
    for i, a in enumerate((h, m, l, h, m, l, h, m)):
        rhs[3 * i: 3 * i + 3] = a
    rhs[24:28] = np.ones((4, npts), dtype=BF16)
    for i in range(4):
        rhs[28 + i] = sqp[i]
    return lhs, rhs


# -------------------------------------------------------------- device build

def _build_nc():
    nc = bacc.Bacc("TRN2", target_bir_lowering=False, debug=False)
    lhs_d = nc.dram_tensor("lhs", [NDIR, K, N], mybir.dt.bfloat16,
                           kind="ExternalInput")
    rhs_d = nc.dram_tensor("rhs", [NDIR, K, N], mybir.dt.bfloat16,
                           kind="ExternalInput")
    rm_d = nc.dram_tensor("rowmins", [128, RM_COLS], mybir.dt.float32,
                          kind="ExternalOutput")
    lhs_ap, rhs_ap, rm_ap = lhs_d.ap(), rhs_d.ap(), rm_d.ap()

    with tile.TileContext(nc) as tc, ExitStack() as ctx:
        stacks = ctx.enter_context(tc.tile_pool(name="stacks", bufs=1))
        psum = ctx.enter_context(
            tc.tile_pool(name="psum", bufs=2, space="PSUM"))
        qpool = ctx.enter_context(tc.tile_pool(name="qcopy", bufs=3))
        spool = ctx.enter_context(tc.tile_pool(name="scratch", bufs=3))
        rmpool = ctx.enter_context(tc.tile_pool(name="rm", bufs=1))

        lhs_t, rhs_t = [], []
        for s in range(NDIR):
            lt = stacks.tile([K, N], mybir.dt.bfloat16, tag=f"lhs{s}")
            nc.sync.dma_start(lt[:], lhs_ap[s])
            rt = stacks.tile([K, N], mybir.dt.bfloat16, tag=f"rhs{s}")
            nc.sync.dma_start(rt[:], rhs_ap[s])
            lhs_t.append(lt)
            rhs_t.append(rt)

        rm = rmpool.tile([128, RM_COLS], mybir.dt.float32)

        for s in range(NDIR):
            lt = lhs_t[s]
            rt = rhs_t[s ^ 1]          # query side s pairs with the other tensor
            for c in range(NCHUNK):
                lhsT = lt[:, c * 128:(c + 1) * 128]
                pa = psum.tile([128, HALF], mybir.dt.float32, tag="ps")
                pb = psum.tile([128, HALF], mybir.dt.float32, tag="ps")
                for j in range(4):
                    nc.tensor.matmul(
                        pa[:, j * 512:(j + 1) * 512], lhsT,
                        rt[:, j * 512:(j + 1) * 512])
                for j in range(4):
                    nc.tensor.matmul(
                        pb[:, j * 512:(j + 1) * 512], lhsT,
                        rt[:, HALF + j * 512: HALF + (j + 1) * 512])
                q = qpool.tile([128, HALF], mybir.dt.float32, tag="q")
                nc.scalar.copy(q[:], pa[:])
                sc = spool.tile([128, HALF], mybir.dt.float32, tag="sc")
                # running min over interleaved (q, pb) pairs: the last output
                # column is the full row-min of the [128, 4096] stripe.
                nc.vector.tensor_tensor_scan(
                    out=sc[:], data0=q[:], data1=pb[:], initial=BIG,
                    op0=mybir.AluOpType.min, op1=mybir.AluOpType.min)
                col = s * NCHUNK + c
                nc.scalar.copy(rm[:, col:col + 1], sc[:, HALF - 1:HALF])
        nc.sync.dma_start(rm_ap, rm[:])
    nc.compile()
    return nc


_CACHE: dict = {}


def _get_nc():
    if "nc" not in _CACHE:
        _CACHE["nc"] = _build_nc()
    return _CACHE["nc"]


# --------------------------------------------------------------------- entry

def make_in_maps(xyz1, xyz2):
    in_maps = []
    for core in range(NCORES):
        lhs = np.empty((NDIR, K, N), dtype=BF16)
        rhs = np.empty((NDIR, K, N), dtype=BF16)
        for bl in range(BPC):
            b = core * BPC + bl
            for t, z in ((0, xyz1[b]), (1, xyz2[b])):
                lhs[bl * 2 + t], rhs[bl * 2 + t] = _stacks(np.asarray(z))
        in_maps.append({"lhs": lhs, "rhs": rhs})
    return in_maps


def combine(results):
    total = 0.0
    for r in results:
        rm = r["rowmins"].astype(np.float64)
        total += np.maximum(rm, 0.0).sum()
    return np.float32(total / (B * N))


def kernel(xyz1, xyz2, **_):
    in_maps = make_in_maps(xyz1, xyz2)
    res = run_bass_kernel_spmd(_get_nc(), in_maps, core_ids=list(range(NCORES)))
    return combine(res.results)
